# revision 1
# baseline (speedup 1.0000x reference)
"""Trainium2 Bass kernel for nn_Attention_54614804136573 (topk_masking).

Sharding: 8 cores = 4 batches x 2 head-groups (8 heads each). Each core gets
its batch's full x (columns rotated so its own 8 head-chunks come first),
computes the token-importance mask redundantly, runs its 8 heads of attention,
and produces a partial to_out product for its 1024-wide d-slice. The host sums
the two partials per batch and adds bo.
"""

import sys

sys.path.insert(0, "/opt/trn_rl_repo")

import numpy as np
import ml_dtypes

import concourse.mybir as mybir
import concourse.tile as tile
from concourse import bacc, bass_utils
from concourse.masks import make_identity
from concourse.tile import add_dep_helper

B = 4
N = 1024
C = 128
D = 2048
NCHUNK = 16  # d-chunks of 128 (= patch positions = heads)
HPC = 8  # heads per core
MASK_NUM = 25
SCALE = 64.0 ** -0.5  # 0.125

F32 = mybir.dt.float32
F32R = mybir.dt.float32r
BF16 = mybir.dt.bfloat16
U32 = mybir.dt.uint32
Exp = mybir.ActivationFunctionType.Exp
Ident = mybir.ActivationFunctionType.Identity
Copy = mybir.ActivationFunctionType.Copy
Copy = mybir.ActivationFunctionType.Copy
NEG_BIG = -1e30


def _body(tc, xc, wq_d, wk_d, wv_d, bq_d, bk_d, bv_d, wtc_d, wo_d, outT_d):
    nc = tc.nc
    mscr = nc.dram_tensor("mscr", (N,), F32, kind="Internal").ap()
    dscr = nc.dram_tensor("dscr", (HPC, N), F32, kind="Internal").ap()

    with (
        tc.tile_pool(name="consts", bufs=1) as consts,
        tc.tile_pool(name="persist", bufs=1) as persist,
    ):
        # ---- constants ----
        ident_ld = consts.tile([128, 128], F32)
        make_identity(nc, ident_ld)
        ident = consts.tile([128, 128], F32R)
        nc.vector.tensor_copy(ident, ident_ld)
        identb = consts.tile([128, 128], BF16)
        make_identity(nc, identb)
        ones_pv = consts.tile([128, 32], BF16)
        nc.vector.memset(ones_pv, 1.0)
        ones_k1_ld = consts.tile([1, 128], F32)
        nc.vector.memset(ones_k1_ld, 1.0)
        ones_k1 = consts.tile([1, 128], F32R)
        nc.vector.tensor_copy(ones_k1, ones_k1_ld)

        # ---- persistent activations ----
        qT = persist.tile([128, HPC, N], F32R)  # [c', h, n] 4 MB
        kT = persist.tile([128, HPC, N], F32R)  # 4 MB
        vnat = persist.tile([128, HPC, 8, C], BF16)  # [j, h, jt, c] 2 MB
        mask_col = persist.tile([128, 8], F32)
        scale_col = persist.tile([128, 8], F32)

        # ================= phase 1: transpose x, logits, mask, QKV =========
        with (
            tc.tile_pool(name="ph1", bufs=2) as ph1,
            tc.tile_pool(name="ph1_vt", bufs=2) as ph1_vt,
            tc.tile_pool(name="ph1big", bufs=1) as ph1big,
            tc.tile_pool(name="mrows", bufs=1) as mrows,
            tc.tile_pool(name="tp_psum", bufs=2, space="PSUM") as tp_psum,
            tc.tile_pool(name="mm_psum", bufs=2, space="PSUM") as mm_psum,
            tc.tile_pool(name="lg_psum", bufs=1, space="PSUM") as lg_psum,
        ):
            xT = ph1big.tile([128, NCHUNK, N], F32R)  # [c, k, n] 8 MB

            for nt in range(8):
                x_nat = ph1.tile([128, D], F32R)
                eng = nc.sync if nt % 2 == 0 else nc.scalar
                step = D // 2
                for dh in range(2):
                    eng.dma_start(
                        out=x_nat[:, dh * step : (dh + 1) * step],
                        in_=xc[nt * 128 : (nt + 1) * 128,
                               dh * step : (dh + 1) * step],
                    )
                for kg in range(4):
                    pt4 = tp_psum.tile([128, 4, 128], F32R, tag="pt4")
                    for dk in range(4):
                        k = kg * 4 + dk
                        nc.tensor.transpose(
                            pt4[:, dk, :], x_nat[:, k * 128 : (k + 1) * 128], ident
                        )
                    dst = xT[:, kg * 4 : kg * 4 + 4, nt * 128 : (nt + 1) * 128]
                    if nt % 2 == 0:
                        nc.vector.tensor_copy(dst, pt4)
                    else:
                        nc.scalar.activation(out=dst, in_=pt4, func=Copy)

            # weight loads: emitted after x so they don't block the x queues
            wq_ld = consts.tile([C, C], F32)
            nc.scalar.dma_start(out=wq_ld, in_=wq_d)
            wq_sb = consts.tile([C, C], F32R)
            nc.vector.tensor_copy(wq_sb, wq_ld)
            wk_ld = consts.tile([C, C], F32)
            nc.scalar.dma_start(out=wk_ld, in_=wk_d)
            wk_sb = consts.tile([C, C], F32R)
            nc.vector.tensor_copy(wk_sb, wk_ld)
            wv_ld = consts.tile([C, C], F32)
            nc.scalar.dma_start(out=wv_ld, in_=wv_d)
            wv_sb = consts.tile([C, C], F32R)
            nc.vector.tensor_copy(wv_sb, wv_ld)
            bq_sb = consts.tile([C, 1], F32)
            nc.scalar.dma_start(out=bq_sb, in_=bq_d)
            bk_sb = consts.tile([C, 1], F32)
            nc.scalar.dma_start(out=bk_sb, in_=bk_d)
            bv_sb = consts.tile([C, 1], F32)
            nc.scalar.dma_start(out=bv_sb, in_=bv_d)
            wtc_ld = consts.tile([C, 1], F32)
            nc.scalar.dma_start(out=wtc_ld, in_=wtc_d)
            wtc_sb = consts.tile([C, 1], F32R)
            nc.vector.tensor_copy(wtc_sb, wtc_ld)

            # logits[n] = sum_k xT[:, k, n] . wtc   (wtc = (Wl@Wq)/16)
            lg = lg_psum.tile([1, N], F32)
            for half in range(2):
                for k in range(NCHUNK):
                    nc.tensor.matmul(
                        lg[:, half * 512 : (half + 1) * 512],
                        wtc_sb,
                        xT[:, k, half * 512 : (half + 1) * 512],
                        start=(k == 0),
                        stop=(k == NCHUNK - 1),
                    )

            # ---- mask: softmax over tokens + snap all but 25 smallest to 1
            smrow = mrows.tile([1, N], F32)
            ssum = mrows.tile([1, 1], F32)
            nc.scalar.activation(out=smrow, in_=lg, func=Exp, accum_out=ssum)
            srecip = mrows.tile([1, 1], F32)
            nc.vector.reciprocal(srecip, ssum)
            nc.vector.tensor_scalar_mul(smrow, smrow, srecip)
            negrow = mrows.tile([1, N], F32)
            nc.vector.tensor_scalar_mul(negrow, lg, -1.0)
            scratch = mrows.tile([1, N], F32)
            nc.vector.tensor_copy(scratch, negrow)
            m8 = mrows.tile([1, 8], F32)
            for _ in range(3):
                nc.vector.max(out=m8, in_=scratch)
                nc.vector.match_replace(
                    out=scratch, in_to_replace=m8, in_values=scratch,
                    imm_value=NEG_BIG,
                )
            nc.vector.max(out=m8, in_=scratch)  # m8[0,0] = 25th largest of -L
            sel = mrows.tile([1, N], U32)
            nc.vector.tensor_scalar(
                sel, negrow, m8[:, 0:1], None, op0=mybir.AluOpType.is_lt
            )
            onesrow = mrows.tile([1, N], F32)
            nc.vector.memset(onesrow, 1.0)
            nc.vector.copy_predicated(smrow, sel, onesrow)
            # row [1, 1024] -> column-major [128, 8] (partition = token % 128)
            w_inst = nc.sync.dma_start(out=mscr, in_=smrow)
            r_inst = nc.sync.dma_start(
                out=mask_col, in_=mscr.rearrange("(t j) -> j t", j=128)
            )
            add_dep_helper(r_inst.ins, w_inst.ins, sync=True, reason="mask RAW via dram")
            nc.vector.tensor_scalar_mul(scale_col, mask_col, SCALE)

            # ---- Q/K projections (transposed layouts) ----
            for h in range(HPC):
                for w_sb, b_sb, dstT in ((wq_sb, bq_sb, qT), (wk_sb, bk_sb, kT)):
                    pp = mm_psum.tile([128, N], F32)
                    for half in range(2):
                        nc.tensor.matmul(
                            pp[:, half * 512 : (half + 1) * 512],
                            w_sb,
                            xT[:, h, half * 512 : (half + 1) * 512],
                            start=True,
                            stop=True,
                        )
                    nc.scalar.activation(
                        out=dstT[:, h, :], in_=pp, func=Ident, bias=b_sb
                    )
            # ---- V projections + transpose to natural layout (needs mask) --
            for h in range(HPC):
                vT_h = ph1_vt.tile([128, N], BF16)
                pp = mm_psum.tile([128, N], F32)
                for half in range(2):
                    nc.tensor.matmul(
                        pp[:, half * 512 : (half + 1) * 512],
                        wv_sb,
                        xT[:, h, half * 512 : (half + 1) * 512],
                        start=True,
                        stop=True,
                    )
                nc.scalar.activation(out=vT_h, in_=pp, func=Ident, bias=bv_sb)
                for jtg in range(2):
                    pv4 = tp_psum.tile([128, 4, 128], BF16, tag="pt4")
                    for dj in range(4):
                        jt = jtg * 4 + dj
                        nc.tensor.transpose(
                            pv4[:, dj, :], vT_h[:, jt * 128 : (jt + 1) * 128],
                            identb,
                        )
                    mslice = mask_col[:, jtg * 4 : (jtg + 1) * 4]
                    nc.vector.tensor_tensor(
                        out=vnat[:, h, jtg * 4 : (jtg + 1) * 4, :],
                        in0=pv4,
                        in1=mslice.unsqueeze(-1).broadcast_to([128, 4, 128]),
                        op=mybir.AluOpType.mult,
                    )

        # ================= phase 2: attention ==============================
        with tc.tile_pool(name="ph2big", bufs=1) as ph2big:
            outT_sb = ph2big.tile([128, HPC, N], BF16)  # [c, h, i] 2 MB
            woT_sb = ph2big.tile([128, HPC, D], BF16)  # [d, h-chunk, o] 4 MB
            for h in range(HPC):
                nc.sync.dma_start(
                    out=woT_sb[:, h, :], in_=wo_d[h * 128 : (h + 1) * 128, :]
                )

            attn_pools = (
                tc.tile_pool(name="pexp", bufs=8),
                tc.tile_pool(name="dvp", bufs=2),
                tc.tile_pool(name="st_psum", bufs=2, space="PSUM"),
                tc.tile_pool(name="ot_psum", bufs=1, space="PSUM"),
                tc.tile_pool(name="dn_psum", bufs=1, space="PSUM"),
            )
            pexp, dvp, st_psum, ot_psum, dn_psum = (
                p.__enter__() for p in attn_pools
            )
            for h in range(HPC):
                ot = ot_psum.tile([128, N], F32)
                dn4 = dn_psum.tile([128, N], F32, tag="dn")

                def emit_pv(jt, pexp_t, ot=ot, h=h):
                    for half in range(2):
                        nc.tensor.matmul(
                            ot[:, half * 512 : (half + 1) * 512],
                            vnat[:, h, jt, :],
                            pexp_t[:, half * 512 : (half + 1) * 512],
                            start=(jt == 0),
                            stop=(jt == 7),
                        )

                def emit_dens(blk_exps, blk, dn4=dn4):
                    # 4 col-groups back-to-back -> concurrent on the PE array
                    for half in range(2):
                        for jt, pexp_t in blk_exps:
                            g = jt % 4
                            nc.tensor.matmul(
                                dn4[32 * g : 32 * (g + 1),
                                    half * 512 : (half + 1) * 512],
                                ones_pv,
                                pexp_t[:, half * 512 : (half + 1) * 512],
                                start=(blk == 0),
                                stop=(blk == 1),
                                tile_position=(0, 32 * g),
                            )

                pending = None  # (jt, pexp tile) waiting for its PV emission
                blk_exps = []
                for jt in range(8):
                    st = st_psum.tile([128, N], F32)
                    for half in range(2):
                        nc.tensor.matmul(
                            st[:, half * 512 : (half + 1) * 512],
                            kT[:, h, jt * 128 : (jt + 1) * 128],
                            qT[:, h, half * 512 : (half + 1) * 512],
                            start=True,
                            stop=True,
                        )
                    pexp_t = pexp.tile([128, N], BF16)
                    nc.scalar.activation(
                        out=pexp_t, in_=st, func=Exp,
                        scale=scale_col[:, jt : jt + 1],
                    )
                    if jt == 4:
                        emit_dens(blk_exps, 0)
                        blk_exps = []
                    if pending is not None:
                        emit_pv(*pending)
                    pending = (jt, pexp_t)
                    blk_exps.append((jt, pexp_t))
                emit_pv(*pending)
                emit_dens(blk_exps, 1)
                nc.vector.tensor_copy(outT_sb[:, h, :], ot)
                rrow = dvp.tile([1, N], F32)
                dsb4 = dvp.tile([128, N], F32)
                nc.vector.tensor_copy(dsb4, dn4)
                nc.gpsimd.dma_start(out=rrow, in_=dsb4[0:1, :])
                for g in range(1, 4):
                    nc.gpsimd.dma_start(
                        out=rrow, in_=dsb4[32 * g : 32 * g + 1, :],
                        accum_op=mybir.AluOpType.add,
                    )
                nc.vector.reciprocal(rrow, rrow)
                w_i = nc.sync.dma_start(out=dscr[h, :], in_=rrow)
                rb_sb = dvp.tile([128, N], F32)
                r_i = nc.sync.dma_start(
                    out=rb_sb, in_=dscr[h, :].partition_broadcast(128)
                )
                add_dep_helper(r_i.ins, w_i.ins, sync=True, reason="recip RAW")
                nc.vector.tensor_mul(
                    outT_sb[:, h, :], outT_sb[:, h, :], rb_sb
                )

            # ============= phase 4: to_out partial =========================
            # fo shares the st_psum slots (same shape) so Wo accumulation can
            # begin as soon as the last exp frees an ST slot -- no pool
            # transition barrier.
            with tc.tile_pool(name="fout", bufs=3) as fout_pool:
                def finish_oc(oc, fo):
                    for half in range(2):
                        nc.tensor.matmul(
                            fo[:, half * 512 : (half + 1) * 512],
                            woT_sb[:, HPC - 1, oc * 128 : (oc + 1) * 128],
                            outT_sb[:, HPC - 1, half * 512 : (half + 1) * 512],
                            start=False,
                            stop=True,
                        )
                    fout = fout_pool.tile([128, N], F32)
                    nc.vector.tensor_copy(fout, fo)
                    for sh in range(2):
                        eng = nc.sync if sh == 0 else nc.scalar
                        eng.dma_start(
                            out=outT_d[oc * 128 : (oc + 1) * 128,
                                       sh * 512 : (sh + 1) * 512],
                            in_=fout[:, sh * 512 : (sh + 1) * 512],
                        )

                pending_oc = None
                for oc in range(16):
                    fo = st_psum.tile([128, N], F32, tag="st")
                    for half in range(2):
                        for h in range(HPC - 1):
                            nc.tensor.matmul(
                                fo[:, half * 512 : (half + 1) * 512],
                                woT_sb[:, h, oc * 128 : (oc + 1) * 128],
                                outT_sb[:, h, half * 512 : (half + 1) * 512],
                                start=(h == 0),
                                stop=False,
                            )
                    if pending_oc is not None:
                        finish_oc(*pending_oc)
                    pending_oc = (oc, fo)
                finish_oc(*pending_oc)

            for p in reversed(attn_pools):
                p.__exit__(None, None, None)


_CACHE = {}


def _get_module():
    if "nc" in _CACHE:
        return _CACHE["nc"]
    nc = bacc.Bacc("TRN2", target_bir_lowering=False, debug=False, num_devices=8)
    xc = nc.dram_tensor("xc", (N, D), F32R, kind="ExternalInput").ap()
    wq_d = nc.dram_tensor("wqT", (C, C), F32, kind="ExternalInput").ap()
    wk_d = nc.dram_tensor("wkT", (C, C), F32, kind="ExternalInput").ap()
    wv_d = nc.dram_tensor("wvT", (C, C), F32, kind="ExternalInput").ap()
    bq_d = nc.dram_tensor("bq", (C, 1), F32, kind="ExternalInput").ap()
    bk_d = nc.dram_tensor("bk", (C, 1), F32, kind="ExternalInput").ap()
    bv_d = nc.dram_tensor("bv", (C, 1), F32, kind="ExternalInput").ap()
    wtc_d = nc.dram_tensor("wtc", (C, 1), F32, kind="ExternalInput").ap()
    wo_d = nc.dram_tensor("woT", (HPC * C, D), BF16, kind="ExternalInput").ap()
    outT_d = nc.dram_tensor("outT", (D, N), F32, kind="ExternalOutput").ap()

    with tile.TileContext(nc) as tc:
        _body(tc, xc, wq_d, wk_d, wv_d, bq_d, bk_d, bv_d, wtc_d, wo_d, outT_d)
    nc.compile()
    _CACHE["nc"] = nc
    return nc


def make_in_maps(x, Wq, bq, Wk, bk, Wv, bv, Wl, bl, Wo, bo):
    x = np.ascontiguousarray(np.asarray(x, np.float32))
    Wq = np.asarray(Wq, np.float32)
    Wk = np.asarray(Wk, np.float32)
    Wv = np.asarray(Wv, np.float32)
    Wl = np.asarray(Wl, np.float32)
    Wo = np.asarray(Wo, np.float32)
    we = (Wl[0] @ Wq) / float(NCHUNK)  # (128,) logits weight per chunk
    common = {
        "wqT": np.ascontiguousarray(Wq.T),
        "wkT": np.ascontiguousarray(Wk.T),
        "wvT": np.ascontiguousarray(Wv.T),
        "bq": np.asarray(bq, np.float32).reshape(C, 1),
        "bk": np.asarray(bk, np.float32).reshape(C, 1),
        "bv": np.asarray(bv, np.float32).reshape(C, 1),
        "wtc": we.astype(np.float32).reshape(C, 1),
    }
    woT = np.ascontiguousarray(Wo.T)  # (d, o)
    woT_half = [
        woT[0:1024, :].astype(ml_dtypes.bfloat16),
        woT[1024:2048, :].astype(ml_dtypes.bfloat16),
    ]
    in_maps = []
    for core in range(8):
        b, g = divmod(core, 2)
        xb = x[b]
        xcore = xb if g == 0 else np.ascontiguousarray(
            np.concatenate([xb[:, 1024:], xb[:, :1024]], axis=1)
        )
        in_maps.append({"xc": xcore, "woT": woT_half[g], **common})
    return in_maps


def run_spmd(in_maps, trace=False, **kw):
    nc = _get_module()
    return bass_utils.run_bass_kernel_spmd(
        nc, in_maps, core_ids=list(range(8)), trace=trace, **kw
    )


def gather(results, bo):
    bo = np.asarray(bo, np.float32)
    out = np.empty((B, N, D), np.float32)
    for b in range(B):
        p0 = results[2 * b]["outT"].T
        p1 = results[2 * b + 1]["outT"].T
        out[b] = p0 + p1 + bo
    return out


def kernel(x, Wq, bq, Wk, bk, Wv, bv, Wl, bl, Wo, bo, stage=None, **_unused):
    in_maps = make_in_maps(x, Wq, bq, Wk, bk, Wv, bv, Wl, bl, Wo, bo)
    try:
        res = run_spmd(in_maps)
    except Exception:
        # transient device/runtime hiccup: retry once after a short pause
        import time as _time

        _time.sleep(2.0)
        res = run_spmd(in_maps)
    return gather(res.results, bo)



# revision 31
# speedup vs baseline: 1.4591x; 1.4591x over previous
"""Trainium2 Bass kernel for nn_Attention_54614804136573 (topk_masking).

Sharding: 8 cores = 4 batches x 2 head-groups (8 heads each). Each core gets
its batch's full x (columns rotated so its own 8 head-chunks come first, in
bf16), computes the token-importance mask redundantly, runs its 8 heads of
attention, and produces a partial to_out product for its 1024-wide d-slice.
The host sums the two partials per batch and adds bo.

Key structural choices (vs the straightforward formulation):
- x arrives bf16 and is transposed by the DMA xbar (dma_start_transpose), so
  the PE array never spends cycles on data-movement transposes.
- K projection is eliminated: scores = x^T (Wq^T Wk) x, with the single
  projected operand qg = (Wq^T Wk)^T-applied to xT. Q/K biases fold away:
  the per-query term is softmax-shift-invariant, the per-key term
  kcol_j = (Wk^T bq).x_j enters the exp() bias operand.
- The top-25 mask is computed with gpsimd kth_largest on the [128, 8]
  column layout; masked tokens get exactly 0 (their softmax value ~1e-3
  contributes ~1e-5 relative, far below tolerance).
- Softmax denominators come from near-free ap=1 matmuls (stationary = pexp
  tile, moving = a ones column), accumulated in PSUM across key tiles.
"""

import sys

sys.path.insert(0, "/opt/trn_rl_repo")

import numpy as np
import ml_dtypes

import concourse.mybir as mybir
import concourse.tile as tile
from concourse import bacc, bass_utils
from concourse.masks import make_identity
from concourse.tile import add_dep_helper

B = 4
N = 1024
C = 128
D = 2048
NCHUNK = 16
HPC = 8  # heads (= 128-wide d-chunks) per core
MASK_NUM = 25
SCALE = 64.0 ** -0.5  # 0.125

F32 = mybir.dt.float32
F32R = mybir.dt.float32r
BF16 = mybir.dt.bfloat16
Exp = mybir.ActivationFunctionType.Exp
Ident = mybir.ActivationFunctionType.Identity
Mult = mybir.AluOpType.mult
IsLt = mybir.AluOpType.is_lt


def _body(tc, xc, wpack_d, bv_d, wo_d, outT_d):
    nc = tc.nc
    import concourse.bass_isa as bass_isa

    with (
        tc.tile_pool(name="consts", bufs=1) as consts,
        tc.tile_pool(name="persist", bufs=1) as persist,
        tc.tile_pool(name="rows", bufs=2) as rows,
        tc.tile_pool(name="vtp", bufs=2) as vtp,
        tc.tile_pool(name="pexp", bufs=4) as pexp,
        tc.tile_pool(name="rbp", bufs=2) as rbp,
        tc.tile_pool(name="fop", bufs=3) as fop,
        tc.tile_pool(name="tiny", bufs=2) as tiny,
        tc.tile_pool(name="bigp", bufs=2, space="PSUM") as bigp,
        tc.tile_pool(name="otp", bufs=1, space="PSUM") as otp,
        tc.tile_pool(name="smp", bufs=1, space="PSUM") as smp,
        tc.tile_pool(name="foap", bufs=1, space="PSUM") as foap,
    ):
        # ---- constants / weights ----
        ident = consts.tile([128, 128], F32)
        make_identity(nc, ident)
        ones_bf = consts.tile([128, 1], BF16)
        nc.vector.memset(ones_bf, 1.0)
        # all small weights packed into one DMA (HWDGE sem lanes are a
        # scarce resource early on; see tile_sem_assignment round-robin).
        # bf16 throughout: walrus rejects mixed 32/16-bit matmul operands.
        wpack_sb = consts.tile([C, 258], BF16)
        nc.scalar.dma_start(out=wpack_sb, in_=wpack_d)
        g_sb = wpack_sb[:, 0:128]
        wvT_sb = wpack_sb[:, 128:256]
        wtc_sb = wpack_sb[:, 256:257]
        w2_sb = wpack_sb[:, 257:258]
        bv_sb = consts.tile([C, 1], F32)
        nc.scalar.dma_start(out=bv_sb, in_=bv_d)
        # warm the exp activation table while everything else loads
        junk = consts.tile([128, 8], F32)
        nc.vector.memset(junk, 0.0)
        nc.scalar.activation(out=junk, in_=junk, func=Exp)

        # ---- persistent activations ----
        GRP0 = [0, 2, 6, 10, 14, 16]
        xTv = [
            persist.tile([128, GRP0[i + 1] - GRP0[i], N], BF16,
                         name=f"xTg{i}")
            for i in range(5)
        ]  # [c, k-in-group, n]

        def xT(k):
            gi = max(i for i in range(5) if GRP0[i] <= k)
            return xTv[gi][:, k - GRP0[gi], :]
        qg = persist.tile([128, HPC, N], BF16)  # [c, h, i]
        foA = persist.tile([128, 16, N], BF16)  # to_out partial, heads 0-3
        vnat = [
            persist.tile([128, 8, C], BF16, name=f"vnat{h}") for h in range(HPC)
        ]  # per head: [j, jt, c]
        woT_sb = persist.tile([128, HPC, D], BF16)  # [c-in-chunk, h, oc]
        outT = persist.tile([128, HPC, N], BF16)  # [c, h, i]
        kcol_sb = persist.tile([128, HPC, 8], F32)
        ebias = persist.tile([128, HPC, 8], F32)
        neglg = persist.tile([128, 8], F32)
        thr2 = persist.tile([1, 2], F32)
        thrb = persist.tile([128, 1], F32)
        scale_col = persist.tile([128, 8], F32)

        # ---- x load+transpose via DMA xbar ----
        # group sizes [2,4,4,4,2]: a small first group gets the PE started
        # ~2us earlier; totals are unchanged
        GRP = [(0, 2), (2, 4), (6, 4), (10, 4), (14, 2)]
        for gi, (k0, nk) in enumerate(GRP):
            nc.sync.dma_start_transpose(
                xTv[gi][:, :, :],
                xc[:, k0 * 128 : (k0 + nk) * 128],
            )

        # ---- logits, directly in [token%128, token//128] column layout:
        # lgcol[:, t] += xT(k)[:, t-block]^T @ wtc  (ap=1 matmuls, ~free) ----
        lg = otp.tile([128, N], F32, tag="ot")
        lgcol = lg[:, 0:8]

        def lg_chunks(ks, last=False):
            for k in ks:
                for t in range(8):
                    # start only once per PSUM bank: start_tensor_calc
                    # zeroes the whole 2KB zero-region, so later column
                    # groups must rely on the pending-zero first-write
                    nc.tensor.matmul(
                        lgcol[:, t : t + 1],
                        xT(k)[:, t * 128 : (t + 1) * 128],
                        wtc_sb,
                        start=(k == 0 and t == 0),
                        stop=(last and k == ks[-1]),
                    )

        # kcol[j] = (Wk^T bq) . x_j per head, directly in column layout
        def emit_kcol(hs):
            for h in hs:
                kc = smp.tile([128, 8], F32, tag="sm", name=f"kc{h}")
                for jt in range(8):
                    nc.tensor.matmul(
                        kc[:, jt : jt + 1],
                        xT(h)[:, jt * 128 : (jt + 1) * 128],
                        w2_sb,
                        start=(jt == 0),
                        stop=True,
                    )
                nc.vector.tensor_copy(kcol_sb[:, h, :], kc)

        # qg / vT projections, interleaved with the tail logits chunks
        def emit_qg(h):
            pp = bigp.tile([128, N], F32, tag="big")
            for half in range(2):
                nc.tensor.matmul(
                    pp[:, half * 512 : (half + 1) * 512],
                    g_sb,
                    xT(h)[:, half * 512 : (half + 1) * 512],
                    start=True,
                    stop=True,
                )
            nc.vector.tensor_copy(qg[:, h, :], pp)

        def emit_vt(h):
            pp = bigp.tile([128, N], F32, tag="big")
            for half in range(2):
                nc.tensor.matmul(
                    pp[:, half * 512 : (half + 1) * 512],
                    wvT_sb,
                    xT(h)[:, half * 512 : (half + 1) * 512],
                    start=True,
                    stop=True,
                )
            vt_row = vtp.tile([128, N], BF16)
            # phase-1 heads bias on Act (done before the exps own it);
            # phase-2-deferred heads bias on DVE
            if h < 4:
                nc.scalar.activation(out=vt_row, in_=pp, func=Ident,
                                     bias=bv_sb)
            else:
                nc.vector.tensor_scalar_add(vt_row, pp, bv_sb)
            nc.sync.dma_start_transpose(vnat[h][:, :, :], vt_row)
            if h >= 4:
                # deferred heads: the mask multiply must follow the transpose
                nc.vector.tensor_tensor(
                    out=vnat[h][:, :, :],
                    in0=vnat[h][:, :, :],
                    in1=scale_col.unsqueeze(-1).broadcast_to([128, 8, C]),
                    op=Mult,
                )

        # Phase-1 ordering principles: (1) every vt bias-copy must clear
        # the Act engine before the mask lands (the exps own Act from then
        # on); (2) the logits tail chunks are emitted with nothing
        # PSUM-slot-blocked in front of them, since the mask gates the
        # whole attention phase; (3) heads 4-7 qg projections migrate into
        # the activation-bound early attention steps.
        lg_chunks([0, 1])
        emit_kcol([0, 1])
        emit_qg(0)
        emit_vt(0)
        emit_qg(1)
        emit_vt(1)
        lg_chunks([2, 3, 4, 5])
        emit_kcol([2, 3, 4, 5])
        emit_qg(2)
        emit_vt(2)
        emit_qg(3)
        emit_vt(3)
        lg_chunks([6, 7])
        emit_kcol([6, 7])
        lg_chunks([8, 9])
        lg_chunks([10, 11])
        lg_chunks([12, 13])
        lg_chunks([14, 15], last=True)

        # ---- mask from the logit columns: kth largest on gpsimd ----
        nc.vector.tensor_scalar_mul(neglg, lgcol, -1.0)
        # threshold midway between the 25th and 26th largest of -logits
        nc.gpsimd.kth_largest(
            thr2, neglg, 8, MASK_NUM, quantile=1.0 - 24.5 / (N - 1.0)
        )
        nc.gpsimd.partition_broadcast(thrb, thr2[0:1, 0:1], 128)
        nc.vector.tensor_scalar(
            scale_col, neglg, thrb[:, 0:1], SCALE, op0=IsLt, op1=Mult
        )
        for h in range(HPC):
            nc.vector.tensor_tensor(
                out=ebias[:, h, :], in0=kcol_sb[:, h, :], in1=scale_col, op=Mult
            )
        # mask the value tiles (per-key-token = per-partition in vnat
        # layout); heads 4-7 are masked inside their deferred emit_vt
        for h in range(4):
            nc.vector.tensor_tensor(
                out=vnat[h][:, :, :],
                in0=vnat[h][:, :, :],
                in1=scale_col.unsqueeze(-1).broadcast_to([128, 8, C]),
                op=Mult,
            )

        # ================= phase 2: attention ==============================
        # Flattened (h, jt) software pipeline: PV/dens for step k are
        # emitted after ST/exp of step k+1, so the PE never waits on the
        # activation engine at head boundaries.
        heads = {}

        def start_head(h):
            ot_t = otp.tile([128, N], F32, tag="ot", name=f"ot{h}")
            # dn occupies the first 8 columns; the recip-transpose target
            # lives in the same bank at [0:8, 8:136]
            dn_t = smp.tile([128, 136], F32, tag="sm", name=f"dn{h}")
            heads[h] = (ot_t, dn_t)

        def emit_pv_dens(h, jt, pexp_t):
            ot, dnt = heads[h]
            dn = dnt[:, 0:8]
            for half in range(2):
                nc.tensor.matmul(
                    ot[:, half * 512 : (half + 1) * 512],
                    vnat[h][:, jt, :],
                    pexp_t[:, half * 512 : (half + 1) * 512],
                    start=(jt == 0),
                    stop=(jt == 7),
                )
            for ib in range(8):
                nc.tensor.matmul(
                    dn[:, ib : ib + 1],
                    pexp_t[:, ib * 128 : (ib + 1) * 128],
                    ones_bf,
                    start=(jt == 0 and ib == 0),
                    stop=(jt == 7),
                )
            if jt == 7:
                finish_head(h)
            # stream the heads-0..3 part of to_out through the exp-bound
            # window of heads 4..7 (one [128,512] tile per pipeline step)
            if h >= 4 and (h, jt) >= (4, 2):
                step = (h - 4) * 8 + jt - 2
                for fi in ([step] if step < 28 else [2 * step - 28,
                                                     2 * step - 27]):
                    oc, sh = divmod(fi, 2)
                    foa = foap.tile([128, 512], F32, tag="foa",
                                    name=f"foa{fi}")
                    for hp in range(4):
                        nc.tensor.matmul(
                            foa,
                            woT_sb[:, hp, oc * 128 : (oc + 1) * 128],
                            outT[:, hp, sh * 512 : (sh + 1) * 512],
                            start=(hp == 0),
                            stop=(hp == 3),
                        )
                    nc.vector.tensor_copy(
                        foA[:, oc, sh * 512 : (sh + 1) * 512], foa
                    )

        def finish_head(h):
            # dens columns -> reciprocal -> row -> broadcast; the raw
            # (unnormalized) PV result is copied out immediately so the ot
            # PSUM frees for the next head, then normalized in place.
            ot, dnt = heads.pop(h)
            recip_sb = tiny.tile([128, 8], F32)
            nc.vector.reciprocal(recip_sb, dnt[:, 0:8])
            rt = dnt[0:8, 8:136]
            nc.tensor.transpose(rt, recip_sb, ident)
            nc.vector.tensor_copy(outT[:, h, :], ot)
            rt_sb = tiny.tile([8, 128], F32, tag="rt")
            nc.vector.tensor_copy(rt_sb, rt)
            rrow = rows.tile([1, N], F32)
            nc.sync.dma_start(out=rrow, in_=rt_sb)
            rb = rbp.tile([128, N], F32)
            nc.gpsimd.partition_broadcast(rb, rrow, 128)
            nc.vector.tensor_tensor(
                out=outT[:, h, :], in0=outT[:, h, :], in1=rb, op=Mult
            )

        pending = None
        for idx in range(HPC * 8):
            h, jt = divmod(idx, 8)
            if jt == 0:
                start_head(h)
            st = bigp.tile([128, N], F32, tag="big")
            for half in range(2):
                nc.tensor.matmul(
                    st[:, half * 512 : (half + 1) * 512],
                    xT(h)[:, jt * 128 : (jt + 1) * 128],
                    qg[:, h, half * 512 : (half + 1) * 512],
                    start=True,
                    stop=True,
                )
            pexp_t = pexp.tile([128, N], BF16)
            exp_i = nc.scalar.activation(
                out=pexp_t,
                in_=st,
                func=Exp,
                scale=scale_col[:, jt : jt + 1],
                bias=ebias[:, h, jt : jt + 1],
            )
            if jt == 4 and h < 4:
                # heads 4-7 score projections, hidden in the exp-bound window
                emit_qg(h + 4)
            if jt == 6 and h < 4:
                emit_vt(h + 4)
            if jt == 0:
                # stream one woT chunk per head; the explicit dep on the
                # head's first exp keeps the scheduler from hoisting these
                # bulk loads in front of the critical x transposes and
                # mask round-trips on the shared DMA engines
                wo_i = nc.gpsimd.dma_start(
                    out=woT_sb[:, h, :], in_=wo_d[h * 128 : (h + 1) * 128, :]
                )
                add_dep_helper(
                    wo_i.ins, exp_i.ins, sync=True, reason="defer woT load"
                )
            if pending is not None:
                emit_pv_dens(*pending)
            pending = (h, jt, pexp_t)
        emit_pv_dens(*pending)

        # ================= phase 3: to_out partial =========================
        def finish_oc(oc, fo):
            # bf16 output halves the writeback; per-half TT keeps the tail
            # DMA from waiting on the full-row add
            fout = fop.tile([128, N], BF16)
            for sh in range(2):
                nc.vector.tensor_tensor(
                    out=fout[:, sh * 512 : (sh + 1) * 512],
                    in0=fo[:, sh * 512 : (sh + 1) * 512],
                    in1=foA[:, oc, sh * 512 : (sh + 1) * 512],
                    op=mybir.AluOpType.add,
                )
                eng = nc.sync if sh == 0 else nc.scalar
                eng.dma_start(
                    out=outT_d[oc * 128 : (oc + 1) * 128,
                               sh * 512 : (sh + 1) * 512],
                    in_=fout[:, sh * 512 : (sh + 1) * 512],
                )

        pending_oc = None
        for oc in range(16):
            fo = bigp.tile([128, N], F32, tag="big")
            # heads 4..7 only (0..3 were accumulated into foA during
            # phase 2); h outer so the last head's operand is needed last
            for h in range(4, HPC):
                for half in range(2):
                    nc.tensor.matmul(
                        fo[:, half * 512 : (half + 1) * 512],
                        woT_sb[:, h, oc * 128 : (oc + 1) * 128],
                        outT[:, h, half * 512 : (half + 1) * 512],
                        start=(h == 4),
                        stop=(h == HPC - 1),
                    )
            if pending_oc is not None:
                finish_oc(*pending_oc)
            pending_oc = (oc, fo)
        finish_oc(*pending_oc)


_CACHE = {}


def _get_module():
    if "nc" in _CACHE:
        return _CACHE["nc"]
    nc = bacc.Bacc("TRN2", target_bir_lowering=False, debug=False, num_devices=8)
    xc = nc.dram_tensor("xc", (N, D), BF16, kind="ExternalInput").ap()
    wpack_d = nc.dram_tensor("wpack", (C, 258), BF16, kind="ExternalInput").ap()
    bv_d = nc.dram_tensor("bv", (C, 1), F32, kind="ExternalInput").ap()
    wo_d = nc.dram_tensor("woT", (HPC * C, D), BF16, kind="ExternalInput").ap()
    outT_d = nc.dram_tensor("outT", (D, N), BF16, kind="ExternalOutput").ap()

    with tile.TileContext(nc) as tc:
        _body(tc, xc, wpack_d, bv_d, wo_d, outT_d)
    nc.compile()
    _CACHE["nc"] = nc
    return nc


def make_in_maps(x, Wq, bq, Wk, bk, Wv, bv, Wl, bl, Wo, bo):
    x = np.ascontiguousarray(np.asarray(x, np.float32))
    Wq = np.asarray(Wq, np.float32)
    Wk = np.asarray(Wk, np.float32)
    Wv = np.asarray(Wv, np.float32)
    Wl = np.asarray(Wl, np.float32)
    Wo = np.asarray(Wo, np.float32)
    bq = np.asarray(bq, np.float32)

    gmat = Wq.T @ Wk  # scores = x^T G x
    w2 = (Wk.T @ bq).reshape(C, 1)  # per-key bias column
    wtc = ((Wl[0] @ Wq) / float(NCHUNK)).reshape(C, 1)  # logits weights
    wpack = np.concatenate([gmat, Wv.T, wtc, w2], axis=1)
    common = {
        "wpack": np.ascontiguousarray(wpack).astype(ml_dtypes.bfloat16),
        "bv": np.asarray(bv, np.float32).reshape(C, 1),
    }
    woT = np.ascontiguousarray(Wo.T)  # (d, o)
    # the V-side mask multiply uses mask*SCALE (saves a pass); Wo absorbs
    # the exact power-of-two compensation factor 1/SCALE = 8
    woT_half = [
        np.ascontiguousarray(woT[0:1024, :] * 8.0).astype(ml_dtypes.bfloat16),
        np.ascontiguousarray(woT[1024:2048, :] * 8.0).astype(ml_dtypes.bfloat16),
    ]
    in_maps = []
    for core in range(8):
        b, g = divmod(core, 2)
        xb = x[b]
        xcore = xb if g == 0 else np.concatenate(
            [xb[:, 1024:], xb[:, :1024]], axis=1
        )
        xcore = np.ascontiguousarray(xcore).astype(ml_dtypes.bfloat16)
        in_maps.append({"xc": xcore, "woT": woT_half[g], **common})
    return in_maps


def run_spmd(in_maps, trace=False, **kw):
    nc = _get_module()
    return bass_utils.run_bass_kernel_spmd(
        nc, in_maps, core_ids=list(range(8)), trace=trace, **kw
    )


def gather(results, bo):
    bo = np.asarray(bo, np.float32)
    out = np.empty((B, N, D), np.float32)
    for b in range(B):
        p0 = np.asarray(results[2 * b]["outT"], np.float32).T
        p1 = np.asarray(results[2 * b + 1]["outT"], np.float32).T
        out[b] = p0 + p1 + bo
    return out


def kernel(x, Wq, bq, Wk, bk, Wv, bv, Wl, bl, Wo, bo, stage=None, **_unused):
    in_maps = make_in_maps(x, Wq, bq, Wk, bk, Wv, bv, Wl, bl, Wo, bo)
    try:
        res = run_spmd(in_maps)
    except Exception:
        # transient device/runtime hiccup: retry once after a short pause
        import time as _time

        _time.sleep(2.0)
        res = run_spmd(in_maps)
    return gather(res.results, bo)


# revision 35
# speedup vs baseline: 1.4982x; 1.0268x over previous
"""Trainium2 Bass kernel for nn_Attention_54614804136573 (topk_masking).

Sharding: 8 cores = 4 batches x 2 head-groups (8 heads each). Each core gets
its batch's full x (columns rotated so its own 8 head-chunks come first, in
bf16), computes the token-importance mask redundantly, runs its 8 heads of
attention, and produces a partial to_out product for its 1024-wide d-slice.
The host sums the two partials per batch and adds bo.

Key structural choices (vs the straightforward formulation):
- x arrives bf16 and is transposed by the DMA xbar (dma_start_transpose), so
  the PE array never spends cycles on data-movement transposes.
- K projection is eliminated: scores = x^T (Wq^T Wk) x, with the single
  projected operand qg = (Wq^T Wk)^T-applied to xT. Q/K biases fold away:
  the per-query term is softmax-shift-invariant, the per-key term
  kcol_j = (Wk^T bq).x_j enters the exp() bias operand.
- The top-25 mask is computed with gpsimd kth_largest on the [128, 8]
  column layout; masked tokens get exactly 0 (their softmax value ~1e-3
  contributes ~1e-5 relative, far below tolerance).
- Softmax denominators come from near-free ap=1 matmuls (stationary = pexp
  tile, moving = a ones column), accumulated in PSUM across key tiles.
"""

import sys

sys.path.insert(0, "/opt/trn_rl_repo")

import numpy as np
import ml_dtypes

import concourse.mybir as mybir
import concourse.tile as tile
from concourse import bacc, bass_utils
from concourse.masks import make_identity
from concourse.tile import add_dep_helper

B = 4
N = 1024
C = 128
D = 2048
NCHUNK = 16
HPC = 8  # heads (= 128-wide d-chunks) per core
MASK_NUM = 25
SCALE = 64.0 ** -0.5  # 0.125

F32 = mybir.dt.float32
F32R = mybir.dt.float32r
BF16 = mybir.dt.bfloat16
Exp = mybir.ActivationFunctionType.Exp
Ident = mybir.ActivationFunctionType.Identity
Mult = mybir.AluOpType.mult
IsLt = mybir.AluOpType.is_lt


def _body(tc, xc, wpack_d, bv_d, wo_d, outT_d):
    nc = tc.nc
    import concourse.bass_isa as bass_isa

    with (
        tc.tile_pool(name="consts", bufs=1) as consts,
        tc.tile_pool(name="persist", bufs=1) as persist,
        tc.tile_pool(name="rows", bufs=2) as rows,
        tc.tile_pool(name="vtp", bufs=2) as vtp,
        tc.tile_pool(name="pexp", bufs=4) as pexp,
        tc.tile_pool(name="rbp", bufs=2) as rbp,
        tc.tile_pool(name="fop", bufs=3) as fop,
        tc.tile_pool(name="tiny", bufs=2) as tiny,
        tc.tile_pool(name="bigp", bufs=2, space="PSUM") as bigp,
        tc.tile_pool(name="otp", bufs=1, space="PSUM") as otp,
        tc.tile_pool(name="smp", bufs=1, space="PSUM") as smp,
        tc.tile_pool(name="foap", bufs=1, space="PSUM") as foap,
    ):
        # ---- constants / weights ----
        ident = consts.tile([128, 128], F32)
        make_identity(nc, ident)
        ones_bf = consts.tile([128, 1], BF16)
        nc.vector.memset(ones_bf, 1.0)
        # all small weights packed into one DMA (HWDGE sem lanes are a
        # scarce resource early on; see tile_sem_assignment round-robin).
        # bf16 throughout: walrus rejects mixed 32/16-bit matmul operands.
        wpack_sb = consts.tile([C, 258], BF16)
        nc.sync.dma_start(out=wpack_sb, in_=wpack_d)
        g_sb = wpack_sb[:, 0:128]
        wvT_sb = wpack_sb[:, 128:256]
        wtc_sb = wpack_sb[:, 256:257]
        w2_sb = wpack_sb[:, 257:258]
        bv_sb = consts.tile([C, 1], F32)
        nc.sync.dma_start(out=bv_sb, in_=bv_d)
        # warm the exp activation table while everything else loads
        junk = consts.tile([128, 8], F32)
        nc.vector.memset(junk, 0.0)
        nc.scalar.activation(out=junk, in_=junk, func=Exp)

        # ---- persistent activations ----
        GRP0 = [0, 2, 6, 10, 14, 16]
        xTv = [
            persist.tile([128, GRP0[i + 1] - GRP0[i], N], BF16,
                         name=f"xTg{i}")
            for i in range(5)
        ]  # [c, k-in-group, n]

        def xT(k):
            gi = max(i for i in range(5) if GRP0[i] <= k)
            return xTv[gi][:, k - GRP0[gi], :]
        qg = persist.tile([128, HPC, N], BF16)  # [c, h, i]
        foA = persist.tile([128, 16, N], BF16)  # to_out partial, heads 0-3
        vnat = [
            persist.tile([128, 8, C], BF16, name=f"vnat{h}") for h in range(HPC)
        ]  # per head: [j, jt, c]
        woT_sb = persist.tile([128, HPC, D], BF16)  # [c-in-chunk, h, oc]
        outT = persist.tile([128, HPC, N], BF16)  # [c, h, i]
        kcol_sb = persist.tile([128, HPC, 8], F32)
        ebias = persist.tile([128, HPC, 8], F32)
        neglg = persist.tile([128, 8], F32)
        thr2 = persist.tile([1, 2], F32)
        thrb = persist.tile([128, 1], F32)
        scale_col = persist.tile([128, 8], F32)

        # ---- x load+transpose via DMA xbar ----
        # group sizes [2,4,4,4,2]: a small first group gets the PE started
        # ~2us earlier; totals are unchanged
        GRP = [(0, 2), (2, 4), (6, 4), (10, 4), (14, 2)]
        for gi, (k0, nk) in enumerate(GRP):
            nc.sync.dma_start_transpose(
                xTv[gi][:, :, :],
                xc[:, k0 * 128 : (k0 + nk) * 128],
            )

        # ---- logits, directly in [token%128, token//128] column layout:
        # lgcol[:, t] += xT(k)[:, t-block]^T @ wtc  (ap=1 matmuls, ~free) ----
        lg = otp.tile([128, N], F32, tag="ot")
        lgcol = lg[:, 0:8]

        def lg_chunks(ks, last=False):
            for k in ks:
                for t in range(8):
                    # start only once per PSUM bank: start_tensor_calc
                    # zeroes the whole 2KB zero-region, so later column
                    # groups must rely on the pending-zero first-write
                    nc.tensor.matmul(
                        lgcol[:, t : t + 1],
                        xT(k)[:, t * 128 : (t + 1) * 128],
                        wtc_sb,
                        start=(k == 0 and t == 0),
                        stop=(last and k == ks[-1]),
                    )

        # kcol[j] = (Wk^T bq) . x_j per head, directly in column layout
        def emit_kcol(hs):
            for h in hs:
                kc = smp.tile([128, 8], F32, tag="sm", name=f"kc{h}")
                for jt in range(8):
                    nc.tensor.matmul(
                        kc[:, jt : jt + 1],
                        xT(h)[:, jt * 128 : (jt + 1) * 128],
                        w2_sb,
                        start=(jt == 0),
                        stop=True,
                    )
                nc.vector.tensor_copy(kcol_sb[:, h, :], kc)

        # qg / vT projections, interleaved with the tail logits chunks
        def emit_qg(h):
            pp = bigp.tile([128, N], F32, tag="big")
            for half in range(2):
                nc.tensor.matmul(
                    pp[:, half * 512 : (half + 1) * 512],
                    g_sb,
                    xT(h)[:, half * 512 : (half + 1) * 512],
                    start=True,
                    stop=True,
                )
            nc.vector.tensor_copy(qg[:, h, :], pp)

        def emit_vt(h):
            pp = bigp.tile([128, N], F32, tag="big")
            for half in range(2):
                nc.tensor.matmul(
                    pp[:, half * 512 : (half + 1) * 512],
                    wvT_sb,
                    xT(h)[:, half * 512 : (half + 1) * 512],
                    start=True,
                    stop=True,
                )
            vt_row = vtp.tile([128, N], BF16)
            # phase-1 heads bias on Act (done before the exps own it);
            # phase-2-deferred heads bias on DVE
            if h < 4:
                nc.scalar.activation(out=vt_row, in_=pp, func=Ident,
                                     bias=bv_sb)
            else:
                nc.vector.tensor_scalar_add(vt_row, pp, bv_sb)
            nc.sync.dma_start_transpose(vnat[h][:, :, :], vt_row)
            if h >= 4:
                # deferred heads: the mask multiply must follow the transpose
                nc.vector.tensor_tensor(
                    out=vnat[h][:, :, :],
                    in0=vnat[h][:, :, :],
                    in1=scale_col.unsqueeze(-1).broadcast_to([128, 8, C]),
                    op=Mult,
                )

        # Phase-1 ordering principles: (1) every vt bias-copy must clear
        # the Act engine before the mask lands (the exps own Act from then
        # on); (2) the logits tail chunks are emitted with nothing
        # PSUM-slot-blocked in front of them, since the mask gates the
        # whole attention phase; (3) heads 4-7 qg projections migrate into
        # the activation-bound early attention steps.
        lg_chunks([0, 1])
        emit_kcol([0, 1])
        emit_qg(0)
        emit_vt(0)
        emit_qg(1)
        emit_vt(1)
        lg_chunks([2, 3, 4, 5])
        emit_kcol([2, 3, 4, 5])
        emit_qg(2)
        emit_vt(2)
        emit_qg(3)
        emit_vt(3)
        lg_chunks([6, 7])
        emit_kcol([6, 7])
        lg_chunks([8, 9])
        lg_chunks([10, 11])
        lg_chunks([12, 13])
        lg_chunks([14, 15], last=True)

        # ---- mask from the (host-negated) logit columns ----
        # lgcol holds -logits (wtc is negated on the host), so the bottom-25
        # threshold is the midpoint of its 25th/26th largest values
        nc.vector.tensor_copy(neglg, lgcol)
        nc.gpsimd.kth_largest(
            thr2, neglg, 8, MASK_NUM, quantile=1.0 - 24.5 / (N - 1.0)
        )
        nc.gpsimd.partition_broadcast(thrb, thr2[0:1, 0:1], 128)
        nc.vector.tensor_scalar(
            scale_col, neglg, thrb[:, 0:1], SCALE, op0=IsLt, op1=Mult
        )
        for h in range(HPC):
            nc.vector.tensor_tensor(
                out=ebias[:, h, :], in0=kcol_sb[:, h, :], in1=scale_col, op=Mult
            )
        # mask the value tiles (per-key-token = per-partition in vnat
        # layout); heads 4-7 are masked inside their deferred emit_vt
        for h in range(4):
            nc.vector.tensor_tensor(
                out=vnat[h][:, :, :],
                in0=vnat[h][:, :, :],
                in1=scale_col.unsqueeze(-1).broadcast_to([128, 8, C]),
                op=Mult,
            )

        # ================= phase 2: attention ==============================
        # Flattened (h, jt) software pipeline: PV/dens for step k are
        # emitted after ST/exp of step k+1, so the PE never waits on the
        # activation engine at head boundaries.
        heads = {}

        def start_head(h):
            ot_t = otp.tile([128, N], F32, tag="ot", name=f"ot{h}")
            # dn occupies the first 8 columns; the recip-transpose target
            # lives in the same bank at [0:8, 8:136]
            dn_t = smp.tile([128, 136], F32, tag="sm", name=f"dn{h}")
            heads[h] = (ot_t, dn_t)

        def emit_pv_dens(h, jt, pexp_t):
            ot, dnt = heads[h]
            dn = dnt[:, 0:8]
            for half in range(2):
                nc.tensor.matmul(
                    ot[:, half * 512 : (half + 1) * 512],
                    vnat[h][:, jt, :],
                    pexp_t[:, half * 512 : (half + 1) * 512],
                    start=(jt == 0),
                    stop=(jt == 7),
                )
            for ib in range(8):
                nc.tensor.matmul(
                    dn[:, ib : ib + 1],
                    pexp_t[:, ib * 128 : (ib + 1) * 128],
                    ones_bf,
                    start=(jt == 0 and ib == 0),
                    stop=(jt == 7),
                )
            if jt == 7:
                finish_head(h)
            # stream the heads-0..3 part of to_out through the exp-bound
            # window of heads 4..7 (one [128,512] tile per pipeline step)
            if h >= 4 and (h, jt) >= (4, 2):
                step = (h - 4) * 8 + jt - 2
                for fi in ([step] if step < 28 else [2 * step - 28,
                                                     2 * step - 27]):
                    oc, sh = divmod(fi, 2)
                    foa = foap.tile([128, 512], F32, tag="foa",
                                    name=f"foa{fi}")
                    for hp in range(4):
                        nc.tensor.matmul(
                            foa,
                            woT_sb[:, hp, oc * 128 : (oc + 1) * 128],
                            outT[:, hp, sh * 512 : (sh + 1) * 512],
                            start=(hp == 0),
                            stop=(hp == 3),
                        )
                    nc.vector.tensor_copy(
                        foA[:, oc, sh * 512 : (sh + 1) * 512], foa
                    )

        def finish_head(h):
            # dens columns -> reciprocal -> row -> broadcast; the raw
            # (unnormalized) PV result is copied out immediately so the ot
            # PSUM frees for the next head, then normalized in place.
            ot, dnt = heads.pop(h)
            recip_sb = tiny.tile([128, 8], F32)
            nc.vector.reciprocal(recip_sb, dnt[:, 0:8])
            rt = dnt[0:8, 8:136]
            nc.tensor.transpose(rt, recip_sb, ident)
            nc.vector.tensor_copy(outT[:, h, :], ot)
            rt_sb = tiny.tile([8, 128], F32, tag="rt")
            nc.vector.tensor_copy(rt_sb, rt)
            rrow = rows.tile([1, N], F32)
            nc.sync.dma_start(out=rrow, in_=rt_sb)
            rb = rbp.tile([128, N], F32)
            nc.gpsimd.partition_broadcast(rb, rrow, 128)
            nc.vector.tensor_tensor(
                out=outT[:, h, :], in0=outT[:, h, :], in1=rb, op=Mult
            )

        pending = None
        for idx in range(HPC * 8):
            h, jt = divmod(idx, 8)
            if jt == 0:
                start_head(h)
            st = bigp.tile([128, N], F32, tag="big")
            for half in range(2):
                nc.tensor.matmul(
                    st[:, half * 512 : (half + 1) * 512],
                    xT(h)[:, jt * 128 : (jt + 1) * 128],
                    qg[:, h, half * 512 : (half + 1) * 512],
                    start=True,
                    stop=True,
                )
            pexp_t = pexp.tile([128, N], BF16)
            exp_i = nc.scalar.activation(
                out=pexp_t,
                in_=st,
                func=Exp,
                scale=scale_col[:, jt : jt + 1],
                bias=ebias[:, h, jt : jt + 1],
            )
            if jt == 4 and h < 4:
                # heads 4-7 score projections, hidden in the exp-bound window
                emit_qg(h + 4)
            if jt == 6 and h < 4:
                emit_vt(h + 4)
            if jt == 0:
                # stream one woT chunk per head; the explicit dep on the
                # head's first exp keeps the scheduler from hoisting these
                # bulk loads in front of the critical x transposes and
                # mask round-trips on the shared DMA engines
                wo_i = nc.gpsimd.dma_start(
                    out=woT_sb[:, h, :], in_=wo_d[h * 128 : (h + 1) * 128, :]
                )
                add_dep_helper(
                    wo_i.ins, exp_i.ins, sync=True, reason="defer woT load"
                )
            if pending is not None:
                emit_pv_dens(*pending)
            pending = (h, jt, pexp_t)
        emit_pv_dens(*pending)

        # ================= phase 3: to_out partial =========================
        def finish_oc(oc, fo):
            # bf16 output halves the writeback; pieces keep the tail DMA
            # from waiting on the full-row add (finer near the end)
            fout = fop.tile([128, N], BF16)
            npc = 2
            w = N // npc
            for sh in range(npc):
                nc.vector.tensor_tensor(
                    out=fout[:, sh * w : (sh + 1) * w],
                    in0=fo[:, sh * w : (sh + 1) * w],
                    in1=foA[:, oc, sh * w : (sh + 1) * w],
                    op=mybir.AluOpType.add,
                )
                eng = nc.sync if sh % 2 == 0 else nc.scalar
                eng.dma_start(
                    out=outT_d[oc * 128 : (oc + 1) * 128, sh * w : (sh + 1) * w],
                    in_=fout[:, sh * w : (sh + 1) * w],
                )

        pending_oc = None
        for oc in range(16):
            fo = bigp.tile([128, N], F32, tag="big")
            # heads 4..7 only (0..3 were accumulated into foA during
            # phase 2); h outer so the last head's operand is needed last
            for h in range(4, HPC):
                for half in range(2):
                    nc.tensor.matmul(
                        fo[:, half * 512 : (half + 1) * 512],
                        woT_sb[:, h, oc * 128 : (oc + 1) * 128],
                        outT[:, h, half * 512 : (half + 1) * 512],
                        start=(h == 4),
                        stop=(h == HPC - 1),
                    )
            if pending_oc is not None:
                finish_oc(*pending_oc)
            pending_oc = (oc, fo)
        finish_oc(*pending_oc)


_CACHE = {}


def _get_module():
    if "nc" in _CACHE:
        return _CACHE["nc"]
    nc = bacc.Bacc("TRN2", target_bir_lowering=False, debug=False, num_devices=8)
    xc = nc.dram_tensor("xc", (N, D), BF16, kind="ExternalInput").ap()
    wpack_d = nc.dram_tensor("wpack", (C, 258), BF16, kind="ExternalInput").ap()
    bv_d = nc.dram_tensor("bv", (C, 1), F32, kind="ExternalInput").ap()
    wo_d = nc.dram_tensor("woT", (HPC * C, D), BF16, kind="ExternalInput").ap()
    outT_d = nc.dram_tensor("outT", (D, N), BF16, kind="ExternalOutput").ap()

    with tile.TileContext(nc) as tc:
        _body(tc, xc, wpack_d, bv_d, wo_d, outT_d)
    nc.compile()
    _CACHE["nc"] = nc
    return nc


def make_in_maps(x, Wq, bq, Wk, bk, Wv, bv, Wl, bl, Wo, bo):
    x = np.ascontiguousarray(np.asarray(x, np.float32))
    Wq = np.asarray(Wq, np.float32)
    Wk = np.asarray(Wk, np.float32)
    Wv = np.asarray(Wv, np.float32)
    Wl = np.asarray(Wl, np.float32)
    Wo = np.asarray(Wo, np.float32)
    bq = np.asarray(bq, np.float32)

    gmat = Wq.T @ Wk  # scores = x^T G x
    w2 = (Wk.T @ bq).reshape(C, 1)  # per-key bias column
    # negated so the device-side columns are -logits (mask needs the
    # 25th largest of the negation; saves a pass)
    wtc = (-(Wl[0] @ Wq) / float(NCHUNK)).reshape(C, 1)
    wpack = np.concatenate([gmat, Wv.T, wtc, w2], axis=1)
    common = {
        "wpack": np.ascontiguousarray(wpack).astype(ml_dtypes.bfloat16),
        "bv": np.asarray(bv, np.float32).reshape(C, 1),
    }
    woT = np.ascontiguousarray(Wo.T)  # (d, o)
    # the V-side mask multiply uses mask*SCALE (saves a pass); Wo absorbs
    # the exact power-of-two compensation factor 1/SCALE = 8
    woT_half = [
        np.ascontiguousarray(woT[0:1024, :] * 8.0).astype(ml_dtypes.bfloat16),
        np.ascontiguousarray(woT[1024:2048, :] * 8.0).astype(ml_dtypes.bfloat16),
    ]
    in_maps = []
    for core in range(8):
        b, g = divmod(core, 2)
        xb = x[b]
        xcore = xb if g == 0 else np.concatenate(
            [xb[:, 1024:], xb[:, :1024]], axis=1
        )
        xcore = np.ascontiguousarray(xcore).astype(ml_dtypes.bfloat16)
        in_maps.append({"xc": xcore, "woT": woT_half[g], **common})
    return in_maps


def run_spmd(in_maps, trace=False, **kw):
    nc = _get_module()
    return bass_utils.run_bass_kernel_spmd(
        nc, in_maps, core_ids=list(range(8)), trace=trace, **kw
    )


def gather(results, bo):
    bo = np.asarray(bo, np.float32)
    out = np.empty((B, N, D), np.float32)
    for b in range(B):
        p0 = np.asarray(results[2 * b]["outT"], np.float32).T
        p1 = np.asarray(results[2 * b + 1]["outT"], np.float32).T
        out[b] = p0 + p1 + bo
    return out


def kernel(x, Wq, bq, Wk, bk, Wv, bv, Wl, bl, Wo, bo, stage=None, **_unused):
    in_maps = make_in_maps(x, Wq, bq, Wk, bk, Wv, bv, Wl, bl, Wo, bo)
    try:
        res = run_spmd(in_maps)
    except Exception:
        # transient device/runtime hiccup: retry once after a short pause
        import time as _time

        _time.sleep(2.0)
        res = run_spmd(in_maps)
    return gather(res.results, bo)
